# revision 1
# baseline (speedup 1.0000x reference)
"""CVQNN classifier kernel for 8 Trainium2 NeuronCores.

Math: the whole quantum circuit collapses to a batch-independent affine map
(S, d) on 128-dim phase space.  Per batch row the heavy work is
    msel' = x @ W2 + d20/2          (W2 = S[rows, :64].T, shape (64, 20))
    out_k = log1p(relu(msel'_x[k]^2 + msel'_p[k]^2 + cov_k/4 - 0.5))
i.e. a (B,64) @ (64,20) matmul + elementwise tail -> (B,10).  Memory bound.

Device layout (per core, R = 125952 rows):
  - host splits x into bf16 hi/lo (x = xh + xl exactly to ~2^-17 rel) and
    packs xstack (128, R) bf16: partitions 0..63 = xh features, 64..127 =
    xl features.  Same DMA bytes as fp32 x, but the PE runs single-pass
    bf16 with FWL weight loads instead of double-pass fp32.
  - per super-block (6144 cols = 48 j-blocks): 1 DMA [128, 6144] bf16
    (12 KB per-partition descriptors).  One matmul per j-block,
    stationary = xstack_j [128, 128], moving = wcat [128, 40] =
    [[Wh, Wl], [Wh, 0]]:
      psum cols 0..19  = xh.Wh + xl.Wh   (K-sum does the hi+lo merge)
      psum cols 20..39 = xh.Wl           (correction, merged on DVE)
    (dropped xl.Wl term ~ 2^-18).  One 4-bank psum tile per super-block,
    12 j-blocks in the first 480 cols of each 512-col bank.
  - tail: t2 = r1 + (r2 + d) on DVE (folds the d-add into the hi/lo
    merge; never two PSUM operands in one op), then square (ACT),
    pair-add + cov-add (DVE), relu + ln(1+.) (ACT).
  - DMA out [128, 480]: per-partition 1920 B contiguous, gpsimd SWDGE
    queue so output generation never queues behind input loads.
"""

import ml_dtypes
import numpy as np

import concourse.bacc as bacc
import concourse.mybir as mybir
import concourse.tile as tile
from concourse.bass_utils import run_bass_kernel_spmd

N = 64          # wires
OUT = 10        # measured wires / classes
NCORES = 8
JBLK = 48                  # matmul j-blocks per full super-block
TILE_W = JBLK * 128        # 6144 xstack cols per full super-block
# 20 full super-blocks + two 12-j tail blocks: minimal padding (0.76%)
# and a short serial drain at the end of the pipeline
WIDTHS = [JBLK] * 20 + [12, 12]
R = 128 * sum(WIDTHS)      # per-core rows = 125952
B_PAD = R * NCORES         # 1007616
F32 = mybir.dt.float32
BF16 = mybir.dt.bfloat16
NPBF16 = ml_dtypes.bfloat16


# ---------------------------------------------------------------- host math
def _bs_pass(n, start, int_params):
    i = np.arange(start, n - 1, 2)
    j = i + 1
    theta = int_params[3 * i]
    phi = int_params[3 * i + 1]
    ct, st = np.cos(theta), np.sin(theta)
    cp, sp = np.cos(phi), np.sin(phi)
    S = np.eye(2 * n)
    S[i, i] = ct
    S[i, j] = -cp * st
    S[i, n + j] = -sp * st
    S[j, i] = cp * st
    S[j, j] = ct
    S[j, n + i] = -sp * st
    S[n + i, j] = sp * st
    S[n + i, n + i] = ct
    S[n + i, n + j] = -cp * st
    S[n + j, i] = sp * st
    S[n + j, n + i] = cp * st
    S[n + j, n + j] = ct
    return S


def _layer_symplectic(n, int1, squeezes, int2):
    M = _bs_pass(n, 0, int1)
    M = _bs_pass(n, 1, int1) @ M
    c = np.concatenate([np.cos(int1[2::3]), np.ones(1)])
    s = np.concatenate([np.sin(int1[2::3]), np.zeros(1)])
    Rm = np.block([[np.diag(c), np.diag(-s)], [np.diag(s), np.diag(c)]])
    Sq = np.diag(np.concatenate([np.exp(-squeezes), np.exp(squeezes)]))
    M = Sq @ (Rm @ M)
    M = _bs_pass(n, 0, int2) @ M
    M = _bs_pass(n, 1, int2) @ M
    return M


def _affine_map(layers):
    n = N
    S = np.eye(2 * n)
    d = np.zeros(2 * n)
    for int1, sq, int2, disp in layers:
        M = _layer_symplectic(n, int1, sq, int2)
        S = M @ S
        d = M @ d
        d[:n] += 2.0 * disp
    return S, d


def _device_constants(layers):
    S, d = _affine_map(layers)
    w = np.arange(OUT)
    rows = np.concatenate([w, N + w])
    cov = S @ S.T
    cov_term = cov[w, w] + cov[N + w, N + w]            # (10,)
    W2 = S[rows, :N].T.astype(np.float32)               # (64, 20), msel' scale
    d20 = (d[rows] / 2.0).astype(np.float32)            # (20,)
    covc = (cov_term / 4.0 - 0.5).astype(np.float32)    # (10,)

    Wh = W2.astype(NPBF16)
    Wl = (W2 - Wh.astype(np.float32)).astype(NPBF16)
    wcat = np.zeros((128, 40), NPBF16)                  # [[Wh, Wl], [Wh, 0]]
    wcat[0:64, 0:20] = Wh
    wcat[0:64, 20:40] = Wl
    wcat[64:128, 0:20] = Wh

    dconst = np.ascontiguousarray(np.broadcast_to(
        np.tile(d20, JBLK), (128, 20 * JBLK))).astype(np.float32)
    cconst = np.ascontiguousarray(np.broadcast_to(
        np.tile(covc, JBLK), (128, 10 * JBLK))).astype(np.float32)
    return wcat, dconst, cconst


# ---------------------------------------------------------------- bass build
def build_nc(widths=None):
    widths = widths or WIDTHS
    rr = 128 * sum(widths)
    nc = bacc.Bacc("TRN2", target_bir_lowering=False)
    WC = 20 * JBLK                             # tw cols per super-block (960)
    OC = 10 * JBLK                             # out cols per super-block (480)
    xs = nc.dram_tensor("xs", (128, rr), BF16, kind="ExternalInput")
    wst = nc.dram_tensor("wcat", (128, 40), BF16, kind="ExternalInput")
    dcon = nc.dram_tensor("dconst", (128, WC), F32, kind="ExternalInput")
    ccon = nc.dram_tensor("covconst", (128, OC), F32, kind="ExternalInput")
    out = nc.dram_tensor("out", (128, (rr // 128) * 10), F32,
                         kind="ExternalOutput")

    Square = mybir.ActivationFunctionType.Square
    Relu = mybir.ActivationFunctionType.Relu
    Ln = mybir.ActivationFunctionType.Ln

    with tile.TileContext(nc) as tc:
        with (
            tc.tile_pool(name="const", bufs=1) as cpool,
            tc.tile_pool(name="xin", bufs=4) as xpool,
            tc.tile_pool(name="mid", bufs=3) as mpool,
            tc.tile_pool(name="ob", bufs=3) as opool,
            tc.tile_pool(name="ps", bufs=2, space="PSUM") as pspool,
        ):
            # w_t gates the first matmul: load it first on the sync queue
            # (the gpsimd queue can race its ucode load during the preamble)
            w_t = cpool.tile([128, 40], BF16)
            nc.sync.dma_start(w_t[:], wst[:])
            d_t = cpool.tile([128, WC], F32)
            nc.gpsimd.dma_start(d_t[:], dcon[:])
            c_t = cpool.tile([128, OC], F32)
            nc.gpsimd.dma_start(c_t[:], ccon[:])

            def emit_sb(col_base, jblk, in_chunks):
                wc, oc, nbank = 20 * jblk, 10 * jblk, jblk // 12
                w = 128 * jblk
                tin = xpool.tile([128, w], BF16, tag="tin")
                q = w // in_chunks
                for c4 in range(in_chunks):
                    nc.sync.dma_start(
                        tin[:, c4 * q:(c4 + 1) * q],
                        xs[:, col_base + c4 * q:col_base + (c4 + 1) * q])

                # psum: 12 j-blocks use the first 480 cols of each 512-col
                # bank (no bank crossing)
                ps = pspool.tile([128, nbank, 512], F32, tag="ps")
                for j in range(jblk):
                    nc.tensor.matmul(
                        ps[:, j // 12, 40 * (j % 12):40 * (j % 12) + 40],
                        tin[:, 128 * j:128 * j + 128], w_t[:],
                        start=True, stop=True,
                    )
                psv = ps[:, :, 0:480].rearrange(
                    "p t (g r k) -> p t g r k", r=2, k=20)
                dv = d_t[:, 0:wc].rearrange(
                    "p (t g k) -> p t g k", t=nbank, k=20)
                # t2 = r1 + (r2 + d): folds the d-add into the hi/lo merge
                t2 = mpool.tile([128, wc], F32, tag="t2")
                u = mpool.tile([128, wc], F32, tag="u")
                uv = u[:].rearrange("p (t g k) -> p t g k", t=nbank, k=20)
                nc.vector.tensor_add(uv, psv[:, :, :, 1, :], dv)
                t2v = t2[:].rearrange("p (t g k) -> p t g k", t=nbank, k=20)
                nc.vector.tensor_add(t2v, psv[:, :, :, 0, :], uv)

                sq = mpool.tile([128, wc], F32, tag="sq")
                nc.scalar.activation(sq[:], t2[:], Square)
                sqg = sq[:].rearrange("p (g k) -> p g k", k=20)
                s = mpool.tile([128, oc], F32, tag="s")
                sv = s[:].rearrange("p (g k) -> p g k", k=10)
                nc.vector.tensor_add(sv, sqg[:, :, 0:10], sqg[:, :, 10:20])
                v = mpool.tile([128, oc], F32, tag="v")
                nc.vector.tensor_add(v[:], s[:], c_t[:, 0:oc])
                r = mpool.tile([128, oc], F32, tag="r")
                nc.scalar.activation(r[:], v[:], Relu)
                o = opool.tile([128, oc], F32, tag="o")
                nc.scalar.activation(o[:], r[:], Ln, bias=1.0)

                ob = (col_base // 128) * 10
                nc.gpsimd.dma_start(out[:, ob:ob + oc], o[:])

            # first tile's DMA in eighths so compute starts sooner
            col = 0
            for i, wdt in enumerate(widths):
                emit_sb(col, wdt, 8 if i == 0 else 1)
                col += 128 * wdt
    nc.compile()
    return nc


# ---------------------------------------------------------------- host glue
def _make_in_maps(x_batch, wcat, dconst, cconst):
    B = x_batch.shape[0]
    xpad = np.zeros((B_PAD, N), np.float32)
    xpad[:B] = x_batch
    xh = xpad.astype(NPBF16)
    xl = (xpad - xh.astype(np.float32)).astype(NPBF16)
    in_maps = []
    for c in range(NCORES):
        sl = slice(c * R, (c + 1) * R)
        xstk = np.empty((128, R), NPBF16)
        xstk[0:64] = xh[sl].T
        xstk[64:128] = xl[sl].T
        in_maps.append({"xs": xstk, "wcat": wcat,
                        "dconst": dconst, "covconst": cconst})
    return in_maps


def _decode_out(results, B):
    full = np.empty((B_PAD, OUT), np.float32)
    for c in range(NCORES):
        O = results[c]["out"].reshape(128, R // 128, OUT)
        rows = O.transpose(1, 0, 2).reshape(R, OUT)
        full[c * R:(c + 1) * R] = rows
    return full[:B]


_NC_CACHE = {}


def kernel(x_batch, int1_0, squeezes_0, int2_0, disp_0,
           int1_1, squeezes_1, int2_1, disp_1, _trace=False):
    layers = [
        (np.asarray(int1_0, np.float64), np.asarray(squeezes_0, np.float64),
         np.asarray(int2_0, np.float64), np.asarray(disp_0, np.float64)),
        (np.asarray(int1_1, np.float64), np.asarray(squeezes_1, np.float64),
         np.asarray(int2_1, np.float64), np.asarray(disp_1, np.float64)),
    ]
    wcat, dconst, cconst = _device_constants(layers)
    in_maps = _make_in_maps(np.asarray(x_batch, np.float32), wcat, dconst, cconst)

    if "nc" not in _NC_CACHE:
        _NC_CACHE["nc"] = build_nc()
    nc = _NC_CACHE["nc"]

    res = run_bass_kernel_spmd(
        nc, in_maps, core_ids=list(range(NCORES)), trace=_trace
    )
    out = _decode_out(res.results, x_batch.shape[0])
    if _trace:
        return out, res
    return out



# revision 3
# speedup vs baseline: 1.0598x; 1.0598x over previous
"""CVQNN classifier kernel for 8 Trainium2 NeuronCores.

Math: the whole quantum circuit collapses to a batch-independent affine map
(S, d) on 128-dim phase space.  Per batch row the heavy work is
    msel' = x @ W2 + d20          (W2 = S[rows, :64].T, shape (64, 20))
    out_k = log1p(relu(msel'_x[k]^2 + msel'_p[k]^2 + cov_k/4 - 0.5))
i.e. a (B,64) @ (64,20) matmul + elementwise tail -> (B,10).  Memory bound:
the only way to go fast is to minimize HBM bytes (fp16 in, fp16 out) and
keep the 16 DMA engines saturated end-to-end.

Device layout (per core, R = 125056 rows = 977 j-blocks of 128):
  - host packs xstack (65, R) fp16: partitions 0..63 = x features
    (transposed), partition 64 = ones.  The ones row turns the d-offset
    into a 65th weight row, so psum = x@W2 + d directly and no separate
    DVE bias-add is needed.  fp16 keeps rel err ~1e-3 (gate is 2e-2) at
    half the DMA bytes of the fp32/hi-lo scheme.
  - per super-block of `jblk` j-blocks: 1 DMA [65, 128*jblk] fp16, then
    one matmul per j-block: stationary = xstack_j [65, 128], moving =
    w65 [65, 20] (rows 0..63 = W2 fp16, row 64 = d20).  psum: 24
    j-blocks per 512-col bank (first 480 cols), up to 4 banks per
    super-block, double-buffered (8 banks total).
  - tail: sq = Square(psum) on ACT (reads PSUM directly, fp16 out),
    s = sq_x + sq_p (DVE), v = s + covc (DVE, fp16), o = ln(1+v) (ACT,
    fp16).  DMA out [128, 10*jblk] fp16 on the DVE HWDGE queue so output
    never serializes behind input loads (and avoids the gpsimd SWDGE
    drain at kernel end).
  - widths taper [24, 48, 96*8, 72, 48, 17]: small first block starts
    compute early; small last blocks shrink the post-DMA pipeline drain.
"""

import ml_dtypes
import numpy as np

import concourse.bacc as bacc
import concourse.mybir as mybir
import concourse.tile as tile
from concourse.bass_utils import run_bass_kernel_spmd

N = 64          # wires
OUT = 10        # measured wires / classes
NCORES = 8
JPB = 24                       # j-blocks per psum bank (24*20 = 480 cols)
WIDTHS = [24, 48] + [96] * 8 + [72, 48, 17]   # j-blocks per super-block
NJ = sum(WIDTHS)               # 977
R = 128 * NJ                   # per-core rows = 125056
B_PAD = R * NCORES             # 1000448
F32 = mybir.dt.float32
F16 = mybir.dt.float16
NPF16 = np.float16


# ---------------------------------------------------------------- host math
def _bs_pass(n, start, int_params):
    i = np.arange(start, n - 1, 2)
    j = i + 1
    theta = int_params[3 * i]
    phi = int_params[3 * i + 1]
    ct, st = np.cos(theta), np.sin(theta)
    cp, sp = np.cos(phi), np.sin(phi)
    S = np.eye(2 * n)
    S[i, i] = ct
    S[i, j] = -cp * st
    S[i, n + j] = -sp * st
    S[j, i] = cp * st
    S[j, j] = ct
    S[j, n + i] = -sp * st
    S[n + i, j] = sp * st
    S[n + i, n + i] = ct
    S[n + i, n + j] = -cp * st
    S[n + j, i] = sp * st
    S[n + j, n + i] = cp * st
    S[n + j, n + j] = ct
    return S


def _layer_symplectic(n, int1, squeezes, int2):
    M = _bs_pass(n, 0, int1)
    M = _bs_pass(n, 1, int1) @ M
    c = np.concatenate([np.cos(int1[2::3]), np.ones(1)])
    s = np.concatenate([np.sin(int1[2::3]), np.zeros(1)])
    Rm = np.block([[np.diag(c), np.diag(-s)], [np.diag(s), np.diag(c)]])
    Sq = np.diag(np.concatenate([np.exp(-squeezes), np.exp(squeezes)]))
    M = Sq @ (Rm @ M)
    M = _bs_pass(n, 0, int2) @ M
    M = _bs_pass(n, 1, int2) @ M
    return M


def _affine_map(layers):
    n = N
    S = np.eye(2 * n)
    d = np.zeros(2 * n)
    for int1, sq, int2, disp in layers:
        M = _layer_symplectic(n, int1, sq, int2)
        S = M @ S
        d = M @ d
        d[:n] += 2.0 * disp
    return S, d


def _device_constants(layers):
    S, d = _affine_map(layers)
    w = np.arange(OUT)
    rows = np.concatenate([w, N + w])
    cov = S @ S.T
    cov_term = cov[w, w] + cov[N + w, N + w]            # (10,)
    W2 = S[rows, :N].T                                  # (64, 20), msel' scale
    d20 = d[rows] / 2.0                                 # (20,)
    covc = (cov_term / 4.0 - 0.5).astype(np.float32)    # (10,)

    w65 = np.zeros((65, 20), NPF16)
    w65[0:64] = W2.astype(NPF16)
    w65[64] = d20.astype(NPF16)

    cconst = np.ascontiguousarray(np.broadcast_to(
        np.tile(covc, 4 * JPB), (128, OUT * 4 * JPB))).astype(NPF16)
    return w65, cconst


# ---------------------------------------------------------------- bass build
def build_nc(widths=None):
    widths = widths or WIDTHS
    nj = sum(widths)
    rr = 128 * nj
    nc = bacc.Bacc("TRN2", target_bir_lowering=False)
    OC = OUT * 4 * JPB                          # out cols per full super-block
    xs = nc.dram_tensor("xs", (65, rr), F16, kind="ExternalInput")
    wst = nc.dram_tensor("w65", (65, 20), F16, kind="ExternalInput")
    ccon = nc.dram_tensor("covconst", (128, OC), F16, kind="ExternalInput")
    out = nc.dram_tensor("out", (128, nj * OUT), F16, kind="ExternalOutput")

    Square = mybir.ActivationFunctionType.Square
    Ln = mybir.ActivationFunctionType.Ln

    with tile.TileContext(nc) as tc:
        with (
            tc.tile_pool(name="const", bufs=1) as cpool,
            tc.tile_pool(name="xin", bufs=4) as xpool,
            tc.tile_pool(name="mid", bufs=3) as mpool,
            tc.tile_pool(name="ob", bufs=3) as opool,
            tc.tile_pool(name="ps", bufs=2, space="PSUM") as pspool,
        ):
            # w_t gates the first matmul: load it first on the sync queue
            w_t = cpool.tile([65, 20], F16)
            nc.sync.dma_start(w_t[:], wst[:])
            c_t = cpool.tile([128, OC], F16)
            nc.scalar.dma_start(c_t[:], ccon[:])

            def emit_sb(col_base, jblk, in_chunks):
                oc = OUT * jblk
                nbank = (jblk + JPB - 1) // JPB
                w = 128 * jblk
                tin = xpool.tile([65, w], F16, tag="tin")
                q = w // in_chunks
                for c4 in range(in_chunks):
                    nc.sync.dma_start(
                        tin[:, c4 * q:(c4 + 1) * q],
                        xs[:, col_base + c4 * q:col_base + (c4 + 1) * q])

                # psum: 24 j-blocks use the first 480 cols of each 512-col
                # bank (no bank crossing)
                ps = pspool.tile([128, nbank, 512], F32, tag="ps")
                for j in range(jblk):
                    nc.tensor.matmul(
                        ps[:, j // JPB, 20 * (j % JPB):20 * (j % JPB) + 20],
                        tin[:, 128 * j:128 * j + 128], w_t[:],
                        start=True, stop=True,
                    )
                # views: psum cols (bank t, group g, r in {x,p}, k in 0..9)
                fullb = jblk // JPB          # banks fully used
                remj = jblk - fullb * JPB    # j-blocks in the last ragged bank
                sq = mpool.tile([128, 20 * jblk], F16, tag="sq")

                def emit_sq(pv, sv):
                    nc.scalar.activation(sv, pv, Square)

                if fullb:
                    pv = ps[:, 0:fullb, 0:20 * JPB]
                    sv = sq[:, 0:20 * fullb * JPB].rearrange(
                        "p (t q) -> p t q", t=fullb)
                    emit_sq(pv, sv)
                if remj:
                    pv = ps[:, fullb, 0:20 * remj]
                    sv = sq[:, 20 * fullb * JPB:20 * jblk]
                    emit_sq(pv, sv)

                sqv = sq[:].rearrange("p (g r k) -> p g r k", r=2, k=OUT)
                s = mpool.tile([128, oc], F16, tag="s")
                sv = s[:].rearrange("p (g k) -> p g k", k=OUT)
                nc.vector.tensor_add(sv, sqv[:, :, 0, :], sqv[:, :, 1, :])
                v = mpool.tile([128, oc], F16, tag="v")
                nc.vector.tensor_add(v[:], s[:], c_t[:, 0:oc])
                o = opool.tile([128, oc], F16, tag="o")
                nc.scalar.activation(o[:], v[:], Ln, bias=1.0)

                ob = (col_base // 128) * OUT
                nc.scalar.dma_start(out[:, ob:ob + oc], o[:])

            # first tile's DMA in quarters so compute starts sooner
            col = 0
            for i, wdt in enumerate(widths):
                emit_sb(col, wdt, 4 if i == 0 else 1)
                col += 128 * wdt
    nc.compile()
    return nc


# ---------------------------------------------------------------- host glue
def _make_in_maps(x_batch, w65, cconst):
    B = x_batch.shape[0]
    xpad = np.zeros((B_PAD, N), NPF16)
    xpad[:B] = x_batch
    in_maps = []
    for c in range(NCORES):
        sl = slice(c * R, (c + 1) * R)
        xstk = np.empty((65, R), NPF16)
        xstk[0:64] = xpad[sl].T
        xstk[64] = 1.0
        in_maps.append({"xs": xstk, "w65": w65, "covconst": cconst})
    return in_maps


def _decode_out(results, B):
    full = np.empty((B_PAD, OUT), np.float32)
    for c in range(NCORES):
        O = results[c]["out"].astype(np.float32).reshape(128, NJ, OUT)
        rows = O.transpose(1, 0, 2).reshape(R, OUT)
        full[c * R:(c + 1) * R] = rows
    return full[:B]


_NC_CACHE = {}


def kernel(x_batch, int1_0, squeezes_0, int2_0, disp_0,
           int1_1, squeezes_1, int2_1, disp_1, _trace=False):
    layers = [
        (np.asarray(int1_0, np.float64), np.asarray(squeezes_0, np.float64),
         np.asarray(int2_0, np.float64), np.asarray(disp_0, np.float64)),
        (np.asarray(int1_1, np.float64), np.asarray(squeezes_1, np.float64),
         np.asarray(int2_1, np.float64), np.asarray(disp_1, np.float64)),
    ]
    w65, cconst = _device_constants(layers)
    in_maps = _make_in_maps(np.asarray(x_batch, np.float32), w65, cconst)

    if "nc" not in _NC_CACHE:
        _NC_CACHE["nc"] = build_nc()
    nc = _NC_CACHE["nc"]

    res = run_bass_kernel_spmd(
        nc, in_maps, core_ids=list(range(NCORES)), trace=_trace
    )
    out = _decode_out(res.results, x_batch.shape[0])
    if _trace:
        return out, res
    return out


# revision 5
# speedup vs baseline: 1.7573x; 1.6582x over previous
"""CVQNN classifier kernel for 8 Trainium2 NeuronCores.

Math: the whole quantum circuit collapses to a batch-independent affine map
(S, d) on 128-dim phase space.  Per batch row the heavy work is
    msel' = x @ W2 + d20          (W2 = S[rows, :64].T, shape (64, 20))
    out_k = log1p(msel'_x[k]^2 + msel'_p[k]^2 + cov_k/4 - 0.5)
i.e. a (B,64) @ (64,20) matmul + elementwise tail -> (B,10).  Memory bound:
minimize HBM bytes (fp16 in, fp16 out; gate is 2e-2, fp16 end-to-end is
~5e-4) and keep the 16 DMA engines saturated end-to-end.

Device layout (per core, R = 125184 rows = 489 pair-blocks of 256):
  - host packs xstack (128, R/2) fp16, "2-pack": column c = (pair b,
    lane l), partitions 0..63 = features of row 256b+l, partitions
    64..127 = features of row 256b+128+l.  Full 128 partitions keeps
    DMA descriptors on all 16 engines (a 65-partition layout was
    measured to use only 13 and run ~25% slower per descriptor) and
    halves the LDWEIGHTS count (one stationary load per 256 rows).
  - per super-block of `jblk` pair-blocks: 1 DMA [128, 128*jblk] fp16
    (12 KB/partition descriptors at jblk=48 — the shape measured at
    ~19.7 B/ns/engine), one matmul per pair-block: stationary =
    xstack_b [128, 128], moving = wcat [128, 40] = [[W2,0],[0,W2]],
    psum cols = [Ax Ap Bx Bp] x 10.  12 pair-blocks per 512-col psum
    bank (480 cols used), up to 4 banks/super-block, double-buffered.
  - tail: t2 = psum + d (DVE, fp16 out), sq = t2^2 (ACT), s = sq_x +
    sq_p (DVE fp16), v = s + covc (DVE fp16), o = ln(1+v) (ACT, fp16).
    relu is dropped: nmean >= 0 exactly (mean photon number), and v is
    a sum of nonnegative fp16 terms so ln(1+v) is always finite.
  - out DMA [128, 20*jblk] fp16 on the scalar HWDGE queue: output never
    queues behind input loads and there is no gpsimd SWDGE drain.
  - widths taper [12, 24, 48*8, 36, 24, 9] pair-blocks: small first
    block starts compute early; small last blocks shrink the post-DMA
    pipeline drain.
"""

import numpy as np

import concourse.bacc as bacc
import concourse.mybir as mybir
import concourse.tile as tile
from concourse.bass_utils import run_bass_kernel_spmd

N = 64          # wires
OUT = 10        # measured wires / classes
NCORES = 8
PPB = 12                       # pair-blocks per psum bank (12*40 = 480 cols)
WIDTHS = [12, 24] + [48] * 8 + [36, 24, 9]    # pair-blocks per super-block
NP2 = sum(WIDTHS)              # 489 pair-blocks
NJ = 2 * NP2                   # 978 j-blocks of 128 rows
R = 128 * NJ                   # per-core rows = 125184
B_PAD = R * NCORES             # 1001472
F32 = mybir.dt.float32
F16 = mybir.dt.float16
NPF16 = np.float16


# ---------------------------------------------------------------- host math
def _bs_pass(n, start, int_params):
    i = np.arange(start, n - 1, 2)
    j = i + 1
    theta = int_params[3 * i]
    phi = int_params[3 * i + 1]
    ct, st = np.cos(theta), np.sin(theta)
    cp, sp = np.cos(phi), np.sin(phi)
    S = np.eye(2 * n)
    S[i, i] = ct
    S[i, j] = -cp * st
    S[i, n + j] = -sp * st
    S[j, i] = cp * st
    S[j, j] = ct
    S[j, n + i] = -sp * st
    S[n + i, j] = sp * st
    S[n + i, n + i] = ct
    S[n + i, n + j] = -cp * st
    S[n + j, i] = sp * st
    S[n + j, n + i] = cp * st
    S[n + j, n + j] = ct
    return S


def _layer_symplectic(n, int1, squeezes, int2):
    M = _bs_pass(n, 0, int1)
    M = _bs_pass(n, 1, int1) @ M
    c = np.concatenate([np.cos(int1[2::3]), np.ones(1)])
    s = np.concatenate([np.sin(int1[2::3]), np.zeros(1)])
    Rm = np.block([[np.diag(c), np.diag(-s)], [np.diag(s), np.diag(c)]])
    Sq = np.diag(np.concatenate([np.exp(-squeezes), np.exp(squeezes)]))
    M = Sq @ (Rm @ M)
    M = _bs_pass(n, 0, int2) @ M
    M = _bs_pass(n, 1, int2) @ M
    return M


def _affine_map(layers):
    n = N
    S = np.eye(2 * n)
    d = np.zeros(2 * n)
    for int1, sq, int2, disp in layers:
        M = _layer_symplectic(n, int1, sq, int2)
        S = M @ S
        d = M @ d
        d[:n] += 2.0 * disp
    return S, d


def _device_constants(layers):
    S, d = _affine_map(layers)
    w = np.arange(OUT)
    rows = np.concatenate([w, N + w])
    cov = S @ S.T
    cov_term = cov[w, w] + cov[N + w, N + w]            # (10,)
    W2 = S[rows, :N].T                                  # (64, 20), msel' scale
    d20 = d[rows] / 2.0                                 # (20,)
    covc = (cov_term / 4.0 - 0.5).astype(np.float32)    # (10,)

    wcat = np.zeros((128, 40), NPF16)                   # [[W2, 0], [0, W2]]
    wcat[0:64, 0:20] = W2.astype(NPF16)
    wcat[64:128, 20:40] = W2.astype(NPF16)

    dconst = np.ascontiguousarray(np.broadcast_to(
        np.tile(d20.astype(np.float32), 2 * 4 * PPB),
        (128, 40 * 4 * PPB))).astype(np.float32)
    cconst = np.ascontiguousarray(np.broadcast_to(
        np.tile(covc, 2 * 4 * PPB), (128, 20 * 4 * PPB))).astype(NPF16)
    return wcat, dconst, cconst


# ---------------------------------------------------------------- bass build
def build_nc(widths=None):
    widths = widths or WIDTHS
    np2 = sum(widths)
    cc = 128 * np2                              # xstack cols
    nc = bacc.Bacc("TRN2", target_bir_lowering=False)
    WC = 40 * 4 * PPB                           # psum cols per full SB (1920)
    OC = 20 * 4 * PPB                           # out cols per full SB (960)
    xs = nc.dram_tensor("xs", (128, cc), F16, kind="ExternalInput")
    wst = nc.dram_tensor("wcat", (128, 40), F16, kind="ExternalInput")
    dcon = nc.dram_tensor("dconst", (128, WC), F32, kind="ExternalInput")
    ccon = nc.dram_tensor("covconst", (128, OC), F16, kind="ExternalInput")
    out = nc.dram_tensor("out", (128, 2 * np2 * OUT), F16,
                         kind="ExternalOutput")

    Square = mybir.ActivationFunctionType.Square
    Ln = mybir.ActivationFunctionType.Ln

    with tile.TileContext(nc) as tc:
        with (
            tc.tile_pool(name="const", bufs=1) as cpool,
            tc.tile_pool(name="xin", bufs=4) as xpool,
            tc.tile_pool(name="mid", bufs=3) as mpool,
            tc.tile_pool(name="ob", bufs=3) as opool,
            tc.tile_pool(name="ps", bufs=2, space="PSUM") as pspool,
        ):
            # w_t gates the first matmul: load it first on the sync queue;
            # d/c consts go on the scalar queue so they don't delay x.
            w_t = cpool.tile([128, 40], F16)
            nc.sync.dma_start(w_t[:], wst[:])
            d_t = cpool.tile([128, WC], F32)
            nc.scalar.dma_start(d_t[:], dcon[:])
            c_t = cpool.tile([128, OC], F16)
            nc.scalar.dma_start(c_t[:], ccon[:])

            def emit_sb(col_base, jblk, in_chunks):
                wc, oc = 40 * jblk, 20 * jblk
                nbank = (jblk + PPB - 1) // PPB
                w = 128 * jblk
                tin = xpool.tile([128, w], F16, tag="tin")
                q = w // in_chunks
                for c4 in range(in_chunks):
                    nc.sync.dma_start(
                        tin[:, c4 * q:(c4 + 1) * q],
                        xs[:, col_base + c4 * q:col_base + (c4 + 1) * q])

                # psum: 12 pair-blocks use the first 480 cols of each
                # 512-col bank (no bank crossing)
                ps = pspool.tile([128, nbank, 512], F32, tag="ps")
                for j in range(jblk):
                    nc.tensor.matmul(
                        ps[:, j // PPB, 40 * (j % PPB):40 * (j % PPB) + 40],
                        tin[:, 128 * j:128 * j + 128], w_t[:],
                        start=True, stop=True,
                    )
                fullb = jblk // PPB          # banks fully used
                remj = jblk - fullb * PPB    # pair-blocks in the ragged bank
                t2 = mpool.tile([128, wc], F16, tag="t2")
                if fullb:
                    pv = ps[:, 0:fullb, 0:40 * PPB]
                    tv = t2[:, 0:40 * fullb * PPB].rearrange(
                        "p (t q) -> p t q", t=fullb)
                    dv = d_t[:, 0:40 * fullb * PPB].rearrange(
                        "p (t q) -> p t q", t=fullb)
                    nc.vector.tensor_add(tv, pv, dv)
                if remj:
                    pv = ps[:, fullb, 0:40 * remj]
                    tv = t2[:, 40 * fullb * PPB:wc]
                    nc.vector.tensor_add(tv, pv, d_t[:, 0:40 * remj])

                sq = mpool.tile([128, wc], F16, tag="sq")
                nc.scalar.activation(sq[:], t2[:], Square)
                sqv = sq[:].rearrange("p (g r k) -> p g r k", r=2, k=OUT)
                s = mpool.tile([128, oc], F16, tag="s")
                sv = s[:].rearrange("p (g k) -> p g k", k=OUT)
                nc.vector.tensor_add(sv, sqv[:, :, 0, :], sqv[:, :, 1, :])
                v = mpool.tile([128, oc], F16, tag="v")
                nc.vector.tensor_add(v[:], s[:], c_t[:, 0:oc])
                o = opool.tile([128, oc], F16, tag="o")
                nc.scalar.activation(o[:], v[:], Ln, bias=1.0)

                ob = (col_base // 128) * 20
                nc.scalar.dma_start(out[:, ob:ob + oc], o[:])

            # first tile's DMA in halves so compute starts sooner
            col = 0
            for i, wdt in enumerate(widths):
                emit_sb(col, wdt, 2 if i == 0 else 1)
                col += 128 * wdt
    nc.compile()
    return nc


# ---------------------------------------------------------------- host glue
def _make_in_maps(x_batch, wcat, dconst, cconst):
    B = x_batch.shape[0]
    xpad = np.zeros((B_PAD, N), NPF16)
    xpad[:B] = x_batch
    in_maps = []
    for c in range(NCORES):
        xc = xpad[c * R:(c + 1) * R]
        # xstk[64*m + f, 128*b + l] = xc[256*b + 128*m + l, f]
        xstk = np.ascontiguousarray(
            xc.reshape(R // 256, 2, 128, N).transpose(1, 3, 0, 2)
            .reshape(128, R // 2))
        in_maps.append({"xs": xstk, "wcat": wcat,
                        "dconst": dconst, "covconst": cconst})
    return in_maps


def _decode_out(results, B):
    full = np.empty((B_PAD, OUT), np.float32)
    for c in range(NCORES):
        O = results[c]["out"].astype(np.float32).reshape(128, NJ, OUT)
        rows = O.transpose(1, 0, 2).reshape(R, OUT)
        full[c * R:(c + 1) * R] = rows
    return full[:B]


_NC_CACHE = {}


def kernel(x_batch, int1_0, squeezes_0, int2_0, disp_0,
           int1_1, squeezes_1, int2_1, disp_1, _trace=False):
    layers = [
        (np.asarray(int1_0, np.float64), np.asarray(squeezes_0, np.float64),
         np.asarray(int2_0, np.float64), np.asarray(disp_0, np.float64)),
        (np.asarray(int1_1, np.float64), np.asarray(squeezes_1, np.float64),
         np.asarray(int2_1, np.float64), np.asarray(disp_1, np.float64)),
    ]
    wcat, dconst, cconst = _device_constants(layers)
    in_maps = _make_in_maps(np.asarray(x_batch, np.float32), wcat, dconst,
                            cconst)

    if "nc" not in _NC_CACHE:
        _NC_CACHE["nc"] = build_nc()
    nc = _NC_CACHE["nc"]

    res = run_bass_kernel_spmd(
        nc, in_maps, core_ids=list(range(NCORES)), trace=_trace
    )
    out = _decode_out(res.results, x_batch.shape[0])
    if _trace:
        return out, res
    return out
